# revision 59
# baseline (speedup 1.0000x reference)
"""Trainium2 Bass kernel for nn_LiveNet — dual Strassen-Winograd variant.

See kernel_verified_108435.py for the shipped GEMM2-only version.  This
variant also applies one Strassen-Winograd level to GEMM1: the W1-side
block operands (A11,A12,S2,S3,S1,S4,A22 in product order) are
precomputed on host and streamed as stationary quad tiles; xT-side
combos T1..T4 run on VectorE; the recombination consumes products
straight out of PSUM (only M1 evicts), with ScalarE applying bias+ReLU
into the resident hT.  GEMM2 reorders its products so the three
direct-hT ones run first, giving the h-side combos a runway.
"""

import os
import sys

import numpy as np

for _p in ("/opt/trn_rl_repo", "/root/.axon_site/_ro/trn_rl_repo"):
    if os.path.isdir(_p) and _p not in sys.path:
        sys.path.append(_p)

import ml_dtypes

import concourse.bacc as bacc
import concourse.bass as bass
import concourse.tile as tile
from concourse import mybir
from concourse.bass_utils import run_bass_kernel_spmd

N_CORES = 8
B, N_IN, N_HID, N_OUT = 4096, 1024, 4096, 1024
BSH = B // N_CORES
P = 128
KT1 = N_IN // P             # 8
MT1 = N_HID // P            # 32
KS = MT1 // 2               # 16
MS = KS                     # 16 m-ptiles per GEMM1 product
NCH = 512
HB = 256

F32 = mybir.dt.float32
BF16 = mybir.dt.bfloat16
RELU = mybir.ActivationFunctionType.Relu
COPY = mybir.ActivationFunctionType.Copy
BF = ml_dtypes.bfloat16

WARMUP = int(os.environ.get("K_WARMUP", "22"))

PROD1 = ["M1", "M2", "M6", "M7", "M5", "M3", "M4"]
PROD2 = ["M1", "M2", "M4", "M6", "M5", "M7", "M3"]
NP_ = 7

# PSUM layout knob: 0 = bank-pair tiles with two col-range groups,
# 1 = one [P, HB] tile per ptile (separate accumulation groups).
PSPLIT = int(os.environ.get("K_PSPLIT", "0"))


def build_nc(reps=1):
    nc = bacc.Bacc("TRN2", target_bir_lowering=False, debug=False,
                   num_devices=N_CORES)

    xtr = nc.declare_dram_parameter("xtr", [P, KT1 * BSH], BF16, isOutput=False)
    a1s = nc.declare_dram_parameter("a1s", [NP_, 4, P, 4 * NCH], BF16,
                                    isOutput=False)
    w2s = nc.declare_dram_parameter("w2s", [NP_, KS, P, NCH], BF16,
                                    isOutput=False)
    b1t = nc.declare_dram_parameter("b1t", [P, MT1], F32, isOutput=False)
    b2v = nc.declare_dram_parameter("b2v", [1, N_OUT], BF16, isOutput=False)
    y = nc.declare_dram_parameter("y", [BSH, N_OUT], BF16, isOutput=True)

    with tile.TileContext(nc) as tc:
        with (
            tc.tile_pool(name="const", bufs=1) as const,
            tc.tile_pool(name="xt", bufs=1) as xt_pool,
            tc.tile_pool(name="ht", bufs=1) as ht_pool,
            tc.tile_pool(name="a1", bufs=8) as a1_pool,
            tc.tile_pool(name="w2", bufs=32) as w2_pool,
            tc.tile_pool(name="scmb", bufs=4) as s_pool,
            tc.tile_pool(name="ug1", bufs=1) as ug1_pool,
            tc.tile_pool(name="tmp", bufs=5) as tmp_pool,
            tc.tile_pool(name="uacc", bufs=1) as u_pool,
            tc.tile_pool(name="yout", bufs=4) as y_pool,
            tc.tile_pool(name="ps", bufs=4, space=bass.MemorySpace.PSUM) as ps_pool,
            tc.tile_pool(name="psb", bufs=4, space=bass.MemorySpace.PSUM) as psb_pool,
        ):
            wz = const.tile([P, P], BF16)
            nc.vector.memset(wz[:], 0.0)
            wps = ps_pool.tile([P, NCH], F32, tag="ps", name="wps")
            for i in range(WARMUP):
                nc.tensor.matmul(wps[:, 0:P], wz[:], wz[:],
                                 start=(i == 0), stop=(i == WARMUP - 1))

            a1_heads = []
            for hq in range(3):
                aqh = a1_pool.tile([P, 4 * NCH], BF16, tag="a1",
                                   name=f"a1h{hq}")
                if hq == 0:
                    nc.scalar.dma_start(out=aqh[:, 0:NCH],
                                        in_=a1s[0, 0, :, 0:NCH])
                    nc.scalar.dma_start(out=aqh[:, NCH:4 * NCH],
                                        in_=a1s[0, 0, :, NCH:4 * NCH])
                else:
                    nc.scalar.dma_start(out=aqh[:], in_=a1s[0, hq])
                a1_heads.append(aqh)
            xt_sb = xt_pool.tile([P, KT1, BSH], BF16, tag="xt", name="xt_sb")
            for q in range(4):
                nc.sync.dma_start(
                    out=xt_sb[:, 2 * q:2 * q + 2, :],
                    in_=xtr[:, 2 * q * BSH:(2 * q + 2) * BSH])

            b1_sb = const.tile([P, MT1], F32)
            nc.gpsimd.dma_start(out=b1_sb[:], in_=b1t[:])
            b2v_sb = const.tile([1, N_OUT], BF16)
            nc.gpsimd.dma_start(out=b2v_sb[:], in_=b2v[:])
            ones_sb = const.tile([1, P], BF16)
            nc.vector.memset(ones_sb[:], 1.0)
            ps_bias = {}
            for nm, off in (("M1", 0), ("M5", NCH)):
                ps_bias[nm] = []
                for p in range(2):
                    psb = psb_pool.tile([P, NCH], F32, tag="psb",
                                        name=f"psb_{nm}_{p}")
                    nc.tensor.matmul(psb[:], ones_sb[:],
                                     b2v_sb[:, off:off + NCH],
                                     start=True, stop=False)
                    ps_bias[nm].append(psb)
            prime1 = const.tile([P, 1], F32)
            nc.scalar.activation(prime1[:], b1_sb[:, 0:1], COPY)
            prime2 = const.tile([1, 1], BF16)
            nc.vector.tensor_copy(prime2[:], b2v_sb[:, 0:1])

            # xT-side GEMM1 combos (T1..T4) on DVE
            t1 = s_pool.tile([P, KS, HB], BF16, tag="sc", name="t1")
            t2 = s_pool.tile([P, KS, HB], BF16, tag="sc", name="t2")
            t3 = s_pool.tile([P, KS, HB], BF16, tag="sc", name="t3")
            t4 = s_pool.tile([P, KS, HB], BF16, tag="sc", name="t4")
            for kk in range(4):
                nc.vector.tensor_sub(t1[:, kk, :], xt_sb[:, kk, HB:BSH],
                                     xt_sb[:, kk, 0:HB])
                nc.vector.tensor_sub(t2[:, kk, :], xt_sb[:, 4 + kk, HB:BSH],
                                     t1[:, kk, :])
                nc.vector.tensor_sub(t3[:, kk, :], xt_sb[:, 4 + kk, HB:BSH],
                                     xt_sb[:, kk, HB:BSH])
                nc.vector.tensor_sub(t4[:, kk, :], t2[:, kk, :],
                                     xt_sb[:, 4 + kk, 0:HB])

            def b1_op(nm, kk):
                if nm == "M1":
                    return xt_sb[:, kk, 0:HB]
                if nm == "M2":
                    return xt_sb[:, 4 + kk, 0:HB]
                if nm == "M3":
                    return xt_sb[:, 4 + kk, HB:BSH]
                return {"M5": t1, "M6": t2, "M7": t3, "M4": t4}[nm][:, kk, :]

            for rep in range(reps):
                ht_sb = ht_pool.tile([P, MT1, BSH], BF16, tag="ht",
                                     name="ht_sb")
                m1g = ug1_pool.tile([P, MS, HB], BF16, tag="m1g", name="m1g")
                m5t = ug1_pool.tile([P, MS, HB], BF16, tag="m5t", name="m5t")
                u2g = ug1_pool.tile([P, MS, HB], BF16, tag="u2g", name="u2g")
                u3g = ug1_pool.tile([P, MS, HB], BF16, tag="u3g", name="u3g")

                w2_tiles = []
                w2_iss = 0

                def issue_w2(cnt):
                    # SP alone (565ns/issue) cannot sustain GEMM2's
                    # 426ns/tile consumption; every 3rd tile issues from
                    # the otherwise-idle gpsimd SWDGE path.
                    nonlocal w2_iss
                    for _ in range(cnt):
                        if w2_iss >= NP_ * KS:
                            return
                        i, kk = w2_iss // KS, w2_iss % KS
                        t = w2_pool.tile([P, NCH], BF16, tag="w2",
                                         name="w2t")
                        eng = nc.gpsimd if (w2_iss % 3 == 2) else nc.sync
                        eng.dma_start(out=t[:], in_=w2s[i, kk])
                        w2_tiles.append(t)
                        w2_iss += 1

                a1_tiles = list(a1_heads)

                def issue_a1(upto):
                    while len(a1_tiles) <= min(upto, NP_ * 4 - 1):
                        j = len(a1_tiles)
                        t = a1_pool.tile([P, 4 * NCH], BF16, tag="a1",
                                         name="a1q")
                        nc.sync.dma_start(out=t[:], in_=a1s[j // 4, j % 4])
                        a1_tiles.append(t)

                # ---- GEMM1 Strassen-Winograd ----
                for i, nm in enumerate(PROD1):
                    for q in range(4):
                        j = i * 4 + q
                        issue_a1(j + 4)
                        if rep == 0 and j % 2 == 0:
                            issue_w2(1)
                        aq = a1_tiles[j]
                        bank = None
                        for mtl in range(4):
                            mt = q * 4 + mtl
                            if mt % 2 == 0:
                                bank = ps_pool.tile([P, 2, HB], F32,
                                                    tag="ps",
                                                    name=f"g1_{nm}_{mt}")
                            half = bank[:, mt % 2, :]
                            for kk in range(4):
                                nc.tensor.matmul(
                                    half,
                                    aq[:, mtl * NCH + kk * P:
                                       mtl * NCH + (kk + 1) * P],
                                    b1_op(nm, kk),
                                    start=(kk == 0),
                                    stop=(kk == 3),
                                )
                            if mt % 2 == 0:
                                continue
                            # consume the finished bank PAIR (mt0, mt0+1):
                            # one [128,512] gating op on DVE/ACT; the
                            # bias-bearing ReLU finals stay per-ptile.
                            m0 = mt - 1
                            pr = bank[:]
                            if nm == "M1":
                                nc.scalar.activation(m1g[:, m0:m0 + 2, :],
                                                     pr, COPY)
                            elif nm == "M2":      # C11 = M1+M2
                                tmp = tmp_pool.tile([P, 2, HB], BF16,
                                                    tag="t", name="c11t")
                                nc.vector.tensor_add(
                                    tmp[:], m1g[:, m0:m0 + 2, :], pr)
                                for h2 in range(2):
                                    nc.scalar.activation(
                                        ht_sb[:, m0 + h2, 0:HB],
                                        tmp[:, h2, :], RELU,
                                        bias=b1_sb[:, m0 + h2:m0 + h2 + 1])
                            elif nm == "M6":
                                nc.vector.tensor_add(
                                    u2g[:, m0:m0 + 2, :],
                                    m1g[:, m0:m0 + 2, :], pr)
                            elif nm == "M7":
                                nc.vector.tensor_add(
                                    u3g[:, m0:m0 + 2, :],
                                    u2g[:, m0:m0 + 2, :], pr)
                            elif nm == "M5":
                                # single gating PSUM read (DVE copy); the
                                # C22 chain runs SBUF-only on gpsimd
                                nc.vector.tensor_copy(m5t[:, m0:m0 + 2, :],
                                                      pr)
                                tmp = tmp_pool.tile([P, 2, HB], BF16,
                                                    tag="t", name="c22t")
                                nc.gpsimd.tensor_add(
                                    tmp[:], u3g[:, m0:m0 + 2, :],
                                    m5t[:, m0:m0 + 2, :])
                                for h2 in range(2):
                                    nc.scalar.activation(   # C22
                                        ht_sb[:, KS + m0 + h2, HB:BSH],
                                        tmp[:, h2, :], RELU,
                                        bias=b1_sb[:, KS + m0 + h2:
                                                    KS + m0 + h2 + 1])
                            elif nm == "M3":
                                # C12 = U2+M5+M3: DVE takes the gating
                                # PSUM read, gpsimd finishes from SBUF
                                tmpa = tmp_pool.tile([P, 2, HB], BF16,
                                                     tag="t", name="c12a")
                                nc.vector.tensor_add(
                                    tmpa[:], u2g[:, m0:m0 + 2, :], pr)
                                tmp = tmp_pool.tile([P, 2, HB], BF16,
                                                    tag="t", name="c12t")
                                nc.gpsimd.tensor_add(
                                    tmp[:], tmpa[:], m5t[:, m0:m0 + 2, :])
                                for h2 in range(2):
                                    nc.scalar.activation(
                                        ht_sb[:, m0 + h2, HB:BSH],
                                        tmp[:, h2, :], RELU,
                                        bias=b1_sb[:, m0 + h2:m0 + h2 + 1])
                            elif nm == "M4":      # C21 = U3-M4
                                tmp = tmp_pool.tile([P, 2, HB], BF16,
                                                    tag="t", name="c21t")
                                nc.vector.tensor_sub(
                                    tmp[:], u3g[:, m0:m0 + 2, :], pr)
                                for h2 in range(2):
                                    nc.scalar.activation(
                                        ht_sb[:, KS + m0 + h2, 0:HB],
                                        tmp[:, h2, :], RELU,
                                        bias=b1_sb[:, KS + m0 + h2:
                                                    KS + m0 + h2 + 1])
                issue_w2(32 - w2_iss)

                # ---- GEMM2 h-side combos ----
                s1 = s_pool.tile([P, KS, HB], BF16, tag="sc", name="s1")
                s2 = s_pool.tile([P, KS, HB], BF16, tag="sc", name="s2")
                s3 = s_pool.tile([P, KS, HB], BF16, tag="sc", name="s3")
                s4 = s_pool.tile([P, KS, HB], BF16, tag="sc", name="s4")
                for kk in range(KS):
                    a11 = ht_sb[:, kk, 0:HB]
                    a21 = ht_sb[:, kk, HB:BSH]
                    a12 = ht_sb[:, KS + kk, 0:HB]
                    a22 = ht_sb[:, KS + kk, HB:BSH]
                    nc.vector.tensor_add(s1[:, kk, :], a21, a22)
                    nc.vector.tensor_sub(s2[:, kk, :], s1[:, kk, :], a11)
                    nc.vector.tensor_sub(s4[:, kk, :], a12, s2[:, kk, :])
                    nc.vector.tensor_sub(s3[:, kk, :], a11, a21)

                # ---- GEMM2 Strassen-Winograd ----
                def a_op(name, kk, p):
                    j0 = p * P
                    if name == "M1":
                        return ht_sb[:, kk, j0:j0 + P]
                    if name == "M2":
                        return ht_sb[:, KS + kk, j0:j0 + P]
                    if name == "M4":
                        return ht_sb[:, KS + kk, HB + j0:HB + j0 + P]
                    s = {"M6": s2, "M5": s1, "M7": s3, "M3": s4}[name]
                    return s[:, kk, j0:j0 + P]

                m1_sb = u_pool.tile([P, 2, NCH], F32, tag="m1", name="m1_sb")
                m4_sb = u_pool.tile([P, 2, NCH], F32, tag="m4", name="m4_sb")
                m5_sb = u_pool.tile([P, 2, NCH], F32, tag="m5", name="m5_sb")
                u2_sb = u_pool.tile([P, 2, NCH], F32, tag="u2", name="u2_sb")
                u3_sb = u_pool.tile([P, 2, NCH], F32, tag="u3", name="u3_sb")
                u4_sb = u_pool.tile([P, 2, NCH], F32, tag="u4", name="u4_sb")

                def emit_y(src0, src1, sub, rows0, col0, p, via_sp=False):
                    y_sb = y_pool.tile([P, NCH], BF16, tag="y", name="y_sb")
                    if sub:
                        nc.vector.tensor_sub(y_sb[:], src0, src1)
                    else:
                        nc.vector.tensor_add(y_sb[:], src0, src1)
                    eng = nc.sync if via_sp else nc.scalar
                    r = rows0 + p * P
                    eng.dma_start(out=y[r:r + P, col0:col0 + NCH],
                                  in_=y_sb[:])

                for i, name in enumerate(PROD2):
                    issue_w2(KS)
                    pss = []
                    for p in range(2):
                        if i == NP_ - 1 and p == 1:
                            break
                        biased = name in ("M1", "M5")
                        if biased:
                            ps2 = ps_bias[name][p]
                        else:
                            ps2 = ps_pool.tile([P, NCH], F32, tag="ps",
                                               name=f"ps_{name}_{p}")
                        for kk in range(KS):
                            nc.tensor.matmul(
                                ps2[:],
                                a_op(name, kk, p),
                                w2_tiles[i * KS + kk][:],
                                start=(kk == 0) and not biased,
                                stop=(kk == KS - 1),
                            )
                        pss.append(ps2)

                    if name == "M1":
                        for p in range(2):
                            nc.scalar.activation(m1_sb[:, p, :], pss[p][:],
                                                 COPY)
                    elif name == "M2":
                        for p in range(2):
                            emit_y(m1_sb[:, p, :], pss[p][:], False,
                                   0, 0, p)
                    elif name == "M4":
                        for p in range(2):
                            nc.scalar.activation(m4_sb[:, p, :], pss[p][:],
                                                 COPY)
                    elif name == "M6":
                        for p in range(2):
                            nc.vector.tensor_add(u2_sb[:, p, :],
                                                 m1_sb[:, p, :], pss[p][:])
                    elif name == "M5":
                        for p in range(2):
                            nc.scalar.activation(m5_sb[:, p, :], pss[p][:],
                                                 COPY)
                        for p in range(2):
                            nc.vector.tensor_add(u4_sb[:, p, :],
                                                 u2_sb[:, p, :],
                                                 m5_sb[:, p, :])
                    elif name == "M7":
                        for p in range(2):
                            nc.vector.tensor_add(u3_sb[:, p, :],
                                                 u2_sb[:, p, :], pss[p][:])
                        for p in range(2):
                            emit_y(u3_sb[:, p, :], m5_sb[:, p, :], False,
                                   HB, NCH, p)
                        for p in range(2):
                            emit_y(u3_sb[:, p, :], m4_sb[:, p, :], True,
                                   HB, 0, p)
                    elif name == "M3":
                        emit_y(u4_sb[:, 0, :], pss[0][:], False, 0, NCH, 0,
                               via_sp=True)
                        for c0, hc in ((0, 3 * NCH // 4),
                                       (3 * NCH // 4, NCH // 4)):
                            psh = ps_pool.tile([P, NCH], F32, tag="ps",
                                               name=f"ps_M3h{c0}")
                            for kk in range(KS):
                                nc.tensor.matmul(
                                    psh[:, 0:hc],
                                    a_op(name, kk, 1),
                                    w2_tiles[i * KS + kk][:, c0:c0 + hc],
                                    start=(kk == 0),
                                    stop=(kk == KS - 1),
                                )
                            y_sb = y_pool.tile([P, hc], BF16, tag="yh",
                                               name="y_sbh")
                            nc.vector.tensor_add(
                                y_sb[:], u4_sb[:, 1, c0:c0 + hc],
                                psh[:, 0:hc],
                            )
                            nc.sync.dma_start(
                                out=y[P:HB, NCH + c0:NCH + c0 + hc],
                                in_=y_sb[:],
                            )
    nc.compile()
    return nc


def _w1_blocks(W1):
    A = np.ascontiguousarray(W1, dtype=np.float32).T
    A11, A12 = A[:2048, :512], A[:2048, 512:]
    A21, A22 = A[2048:, :512], A[2048:, 512:]
    S1 = A21 + A22
    S2 = S1 - A11
    S3 = A11 - A21
    S4 = A12 - S2
    return {"M1": A11, "M2": A12, "M6": S2, "M7": S3, "M5": S1,
            "M3": S4, "M4": A22}


def _prep_shared(W1, b1, W2, b2):
    blocks = _w1_blocks(W1)
    quads = []
    for nm in PROD1:
        Ablk = blocks[nm]
        T4d = Ablk.reshape(16, P, 4, P).transpose(0, 2, 3, 1)
        full = T4d.transpose(0, 2, 1, 3).reshape(16, P, 4 * P)
        quads.append(full.reshape(4, 4, P, 4 * P)
                     .transpose(0, 2, 1, 3).reshape(4, P, 16 * P))
    a1s = np.stack(quads).astype(BF)

    b1t = np.ascontiguousarray(
        np.asarray(b1, dtype=np.float32).reshape(MT1, P).T
    )
    W2 = np.ascontiguousarray(W2, dtype=np.float32)
    B11, B12 = W2[:2048, :NCH], W2[:2048, NCH:]
    B21, B22 = W2[2048:, :NCH], W2[2048:, NCH:]
    T1 = B12 - B11
    T2 = B22 - T1
    T3 = B22 - B12
    T4 = T2 - B21
    bmats = {"M1": B11, "M2": B21, "M4": T4, "M6": T2, "M5": T1,
             "M7": T3, "M3": B22}
    w2s = np.stack([bmats[nm].reshape(KS, P, NCH) for nm in PROD2]
                   ).astype(BF)
    b2 = np.asarray(b2, dtype=np.float32)
    b2v = np.concatenate([b2[:NCH], b2[NCH:] - b2[:NCH]])[None, :].astype(BF)
    return a1s, b1t, w2s, b2v


def kernel(x, W1, b1, W2, b2):
    x = np.ascontiguousarray(x, dtype=np.float32)
    a1s, b1t, w2s, b2v = _prep_shared(W1, b1, W2, b2)

    in_maps = []
    for i in range(N_CORES):
        xs = x[i * BSH:(i + 1) * BSH, :].T.astype(BF)
        xtr_i = np.ascontiguousarray(
            xs.reshape(KT1, P, BSH).transpose(1, 0, 2)
        ).reshape(P, KT1 * BSH)
        in_maps.append(
            {"xtr": xtr_i, "a1s": a1s, "w2s": w2s, "b1t": b1t, "b2v": b2v}
        )

    nc = build_nc()
    res = run_bass_kernel_spmd(nc, in_maps, list(range(N_CORES)))
    y = np.concatenate(
        [np.asarray(res.results[i]["y"]) for i in range(N_CORES)], axis=0
    )
    return y.astype(np.float32)


if __name__ == "__main__":
    rng = np.random.default_rng(0)
    x = rng.standard_normal((B, N_IN), dtype=np.float32)
    W1 = rng.standard_normal((N_IN, N_HID), dtype=np.float32) / 32
    b1 = rng.standard_normal((N_HID,), dtype=np.float32) / 32
    W2 = rng.standard_normal((N_HID, N_OUT), dtype=np.float32) / 64
    b2 = rng.standard_normal((N_OUT,), dtype=np.float32) / 64
    y = kernel(x, W1, b1, W2, b2)
    h = np.maximum(x @ W1 + b1, 0)
    y_ref = h @ W2 + b2
    err = np.linalg.norm(y - y_ref) / np.linalg.norm(y_ref)
    print("rel_l2:", err)
